# revision 32
# baseline (speedup 1.0000x reference)
"""TRN2 Bass kernel for nn_Actor (retrieval_knn).

Data-parallel over batch across 8 NeuronCores (8192 rows/core).
Per core: ap_gather embedding lookup (feature-major), fp32 MLP layer-1 on
TensorE, then scores vs the 2489-entry table with W2 absorbed into the
table side (scores = h @ (table@W2).T + table@b2) as three bf16-split
pairs in one 122-row stacked matmul (fp32-grade precision).

Argmax phase (the bottleneck) avoids full-width DVE Max/MaxIndex passes
(no fast DVE modes) by:
  1. probe: DVE max over the first 249 fp32 score cols -> approx row max mu
  2. Act engine converts PSUM fp32 -> SBUF fp16 with bias=-mu (resolution
     near the top ~ gap*2^-11, validated rel_err ~0.011 on host)
  3. the fp16 [32 blocks x 83] grid is reduced with tensor_tensor max fold
     trees (2x DVE mode for 16-bit): column-max [128,83] and block-max
     [128,32]; MaxIndex runs only on those small arrays.
  4. host decodes idx = b*83 + j.
Tiles are software-pipelined (fold work lags one tile) so Act/PE/DVE
overlap; v16 and wstage are ping-ponged.
"""
import sys
sys.path.insert(0, '/opt/trn_rl_repo')
import numpy as np
import ml_dtypes

B = 65536
NCORES = 8
BC = B // NCORES            # 8192
NW, NPTAB, EMB = 1807, 2490, 10
NPROJ = NPTAB - 1           # 2489 real table entries
NT = 2490                   # padded score width (matmul writes all)
HID = 40
NTILES = BC // 128          # 64
WGRID = 83                  # grid width
NBLK = 30                   # real blocks (30*83 = 2490)
BLKPAD = 32                 # padded block count for clean fold tree
VW = BLKPAD * WGRID         # 2656 fp16 cols in v buffer
CHUNK = 830                 # 10 blocks per matmul/act chunk; 3 chunks = 2490
NCH = 3
PROBE = 249                 # probe width (3 blocks) for the fp32 max shift

_cache = {}


def _bf16(x):
    return np.asarray(x, np.float32).astype(ml_dtypes.bfloat16)


def _build(L=1, hwloop=True):
    from concourse import bacc, mybir, bass
    from concourse.tile import TileContext
    import concourse.mybir as mb
    dt = mybir.dt
    AF = mb.ActivationFunctionType
    nc = bacc.Bacc("TRN2", target_bir_lowering=False, debug=False, num_devices=NCORES)

    widx = nc.dram_tensor("widx", [128, 64], dt.int16, kind="ExternalInput")
    pidx = nc.dram_tensor("pidx", [128, 64], dt.int16, kind="ExternalInput")
    wtab16 = nc.dram_tensor("wtab16", [16, NW], dt.float32, kind="ExternalInput")
    ptab16 = nc.dram_tensor("ptab16", [16, NPTAB], dt.float32, kind="ExternalInput")
    w1t = nc.dram_tensor("w1t", [20, HID], dt.float32, kind="ExternalInput")
    b1e = nc.dram_tensor("b1e", [HID, 1], dt.float32, kind="ExternalInput")
    tstk = nc.dram_tensor("tstk", [122, NT], dt.bfloat16, kind="ExternalInput")
    out_ext = nc.dram_tensor("out", [128, NTILES * 16], dt.uint32, kind="ExternalOutput")

    NIDX = BC // 8           # 1024 ids per gather group
    with TileContext(nc) as tc:
        with tc.tile_pool(name="const", bufs=1) as cp, \
             tc.tile_pool(name="work", bufs=1) as wp, \
             tc.tile_pool(name="sc", bufs=1, space="PSUM") as scp:
            t_wtab = cp.tile([128, NW], dt.float32)
            t_ptab = cp.tile([128, NPTAB], dt.float32)
            t_widx = cp.tile([128, 64], dt.int16)
            t_pidx = cp.tile([128, 64], dt.int16)
            t_w1t = cp.tile([20, HID], dt.float32)
            t_b1 = cp.tile([HID, 1], dt.float32)
            t_tstk = cp.tile([122, NT], dt.bfloat16)
            nc.sync.dma_start(out=t_wtab[0:16, :], in_=wtab16.ap())
            nc.sync.dma_start(out=t_ptab[0:16, :], in_=ptab16.ap())
            nc.sync.dma_start(out=t_widx, in_=widx.ap())
            nc.sync.dma_start(out=t_pidx, in_=pidx.ap())
            nc.sync.dma_start(out=t_w1t, in_=w1t.ap())
            nc.sync.dma_start(out=t_b1, in_=b1e.ap())
            nc.sync.dma_start(out=t_tstk, in_=tstk.ap())
            # replicate gather tables into all 8 groups (doubling)
            for src, n in ((t_wtab, NW), (t_ptab, NPTAB)):
                nc.sync.dma_start(out=src[16:32, :], in_=src[0:16, :])
                nc.sync.dma_start(out=src[32:64, :], in_=src[0:32, :])
                nc.sync.dma_start(out=src[64:128, :], in_=src[0:64, :])

            wg = wp.tile([128, NIDX], dt.float32)
            pg = wp.tile([128, NIDX], dt.float32)
            wgr = wp.tile([128, NIDX], dt.float32r)
            pgr = wp.tile([128, NIDX], dt.float32r)
            x = wp.tile([20, BC], dt.float32r)
            t_w1tr = cp.tile([20, HID], dt.float32r)
            hstack = wp.tile([122, BC], dt.bfloat16)
            h_f32 = wp.tile([HID, BC], dt.float32)
            onesrow = wp.tile([2, BC], dt.bfloat16)
            outbuf = wp.tile([128, NTILES * 16], dt.uint32)
            va = wp.tile([128, VW], dt.float16)
            vb = wp.tile([128, VW], dt.float16)
            vc = wp.tile([128, VW], dt.float16)
            f16a = wp.tile([128, 16, WGRID], dt.float16)
            f16b = wp.tile([128, 8, WGRID], dt.float16)
            f16c = wp.tile([128, 4, WGRID], dt.float16)
            f16d = wp.tile([128, 2, WGRID], dt.float16)
            colmax = wp.tile([128, WGRID], dt.float16)
            wsa = wp.tile([122, 128], dt.bfloat16)
            wsb = wp.tile([122, 128], dt.bfloat16)
            wsc = wp.tile([122, 128], dt.bfloat16)
            m8a = wp.tile([128, 8], dt.float32)
            nmu = wp.tile([128, 1], dt.float32)
            mj8 = wp.tile([128, 8], dt.float16)
            mb8 = wp.tile([128, 8], dt.float16)
            nc.vector.memset(onesrow, 1.0)
            nc.sync.dma_start(out=hstack[120:122, :], in_=onesrow)
            nc.vector.tensor_copy(t_w1tr[:, :], t_w1t[:, :])
            # pad blocks 30..31 of each v buffer stay -big forever
            for v in (va, vb, vc):
                v3p = v.rearrange("p (b w) -> p b w", w=WGRID)
                nc.vector.memset(v3p[:, NBLK:BLKPAD, :], -60000.0)
            # 3 two-bank PSUM chunk buffers; scores use cols 0:830,
            # the MLP reuses them as [40, 1024] blocks.
            psb = [scp.tile([128, 1024], dt.float32, name=f"psb{i}")
                   for i in range(NCH)]

            def stage1a(col, ws):
                """wstage copy + score matmuls for one tile (PE runs ahead).
                col = tile_index*128 (int or iv expression)."""
                nc.scalar.activation(ws, hstack[:, bass.ds(col, 128)], AF.Copy)
                for c in range(NCH):
                    for s0, s1 in ((0, 512), (512, CHUNK)):
                        nc.tensor.matmul(
                            psb[c][:, s0:s1], lhsT=ws,
                            rhs=t_tstk[:, c * CHUNK + s0:c * CHUNK + s1],
                            start=True, stop=True)

            def stage1b(v):
                """probe + shifted fp16 convert for one tile."""
                nc.vector.max(out=m8a, in_=psb[0][:, 0:PROBE])
                nc.scalar.activation(nmu, m8a[:, 0:1], AF.Copy, scale=-1.0)
                for c in range(NCH):
                    nc.scalar.activation(v[:, c * CHUNK:(c + 1) * CHUNK],
                                         psb[c][:, 0:CHUNK],
                                         AF.Identity, bias=nmu)

            def stage2(ob, v):
                """fold trees + small argmaxes, one tile; the largest fold
                ops are offloaded to the (otherwise idle) Pool engine.
                ob = tile_index*16 (int or iv expression)."""
                v3 = v.rearrange("p (b w) -> p b w", w=WGRID)
                tt = nc.vector.tensor_max
                # block fold tree -> colmax -> j*  (GpSimd has no ISA max, so
                # all folds run on DVE at the 2x 16-bit rate)
                tt(f16a[:, 0:8, :], v3[:, 0:8, :], v3[:, 8:16, :])
                tt(f16a[:, 8:16, :], v3[:, 16:24, :], v3[:, 24:32, :])
                tt(f16b[:, :, :], f16a[:, 0:8, :], f16a[:, 8:16, :])
                tt(f16c[:, :, :], f16b[:, 0:4, :], f16b[:, 4:8, :])
                tt(f16d[:, :, :], f16c[:, 0:2, :], f16c[:, 2:4, :])
                tt(colmax[:, :].unsqueeze(1), f16d[:, 0:1, :], f16d[:, 1:2, :])
                nc.vector.max(out=mj8, in_=colmax)
                nc.vector.max_index(out=outbuf[:, bass.ds(ob, 8)],
                                    in_max=mj8, in_values=colmax)
                # width fold (in place): 83 -> 42 -> ... -> 1
                tt(v3[:, :, 0:41], v3[:, :, 0:41], v3[:, :, 42:83])
                tt(v3[:, :, 0:21], v3[:, :, 0:21], v3[:, :, 21:42])
                tt(v3[:, :, 0:10], v3[:, :, 0:10], v3[:, :, 11:21])
                tt(v3[:, :, 0:5], v3[:, :, 0:5], v3[:, :, 6:11])
                tt(v3[:, :, 0:3], v3[:, :, 0:3], v3[:, :, 3:6])
                tt(v3[:, :, 0:1], v3[:, :, 0:1], v3[:, :, 2:3])
                tt(v3[:, :, 0:1], v3[:, :, 0:1], v3[:, :, 1:2])
                bm = v3[:, :, 0:1].squeeze(2)
                nc.vector.max(out=mb8, in_=bm)
                nc.vector.max_index(out=outbuf[:, bass.ds(ob + 8, 8)],
                                    in_max=mb8, in_values=bm)

            for _ in range(L):
                nc.gpsimd.ap_gather(out_ap=wg, in_ap=t_wtab, idxs_ap=t_widx,
                                    channels=128, num_elems=NW, d=1, num_idxs=NIDX)
                nc.gpsimd.ap_gather(out_ap=pg, in_ap=t_ptab, idxs_ap=t_pidx,
                                    channels=128, num_elems=NPTAB, d=1, num_idxs=NIDX)
                # round gathered features to fp32r (full 128 partitions: cheap)
                nc.vector.tensor_copy(wgr[:, :], wg[:, :])
                nc.vector.tensor_copy(pgr[:, :], pg[:, :])
                # x assembly: 16 small DMAs spread across both HWDGE queues
                qs = (nc.sync, nc.scalar)
                for g in range(8):
                    qs[g % 2].dma_start(
                        out=x[0:10, g * NIDX:(g + 1) * NIDX],
                        in_=wgr[16 * g:16 * g + 10, :])
                    qs[(g + 1) % 2].dma_start(
                        out=x[10:20, g * NIDX:(g + 1) * NIDX],
                        in_=pgr[16 * g:16 * g + 10, :])
                # MLP layer 1 (fp32r matmul: 1 cyc/row), 8 chunks of 1024 cols
                # round-robined over the three PSUM buffers; the h1/h2 bf16
                # split runs chunk-wise, alternating DVE/Pool for the sub.
                for j in range(8):
                    sl = slice(j * 1024, (j + 1) * 1024)
                    hm = psb[j % NCH][0:HID, 0:1024]
                    for s0 in (0, 512):
                        nc.tensor.matmul(
                            hm[:, s0:s0 + 512], lhsT=t_w1tr[:, :],
                            rhs=x[:, j * 1024 + s0:j * 1024 + s0 + 512],
                            start=True, stop=True)
                    nc.scalar.activation(h_f32[:, sl], hm,
                                         mb.ActivationFunctionType.Relu, bias=t_b1)
                    nc.vector.tensor_copy(hstack[0:HID, sl], h_f32[:, sl])
                    eng = nc.vector if j % 2 else nc.gpsimd
                    eng.tensor_sub(hstack[64:104, sl], h_f32[:, sl],
                                   hstack[0:HID, sl])
                nc.sync.dma_start(out=hstack[40:64, :], in_=hstack[0:24, :])
                nc.sync.dma_start(out=hstack[104:120, :], in_=hstack[24:40, :])

                # scores + argmax, software-pipelined at chunk granularity
                # with a 3-deep v/ws rotation: per tile slot t emit
                # [probe+Act(t), mm(t+1), folds(t-1)] so Act(t) runs while
                # DVE/Pool fold t-1 and PE trails Act by chunks.
                V = (va, vb, vc)
                WS = (wsa, wsb, wsc)
                stage1a(0, wsa)
                stage1b(va)                          # slot 0
                stage1a(128, wsb)
                stage1b(vb)                          # slot 1
                stage1a(256, wsc)
                stage2(0, va)

                def body(iv):
                    for r in range(3):               # slots 3k+2+r
                        stage1b(V[(2 + r) % 3])
                        stage1a(iv * 384 + 384 + r * 128, WS[r % 3])
                        stage2(iv * 48 + 16 + r * 16, V[(1 + r) % 3])
                if hwloop:
                    with tc.For_i(0, 20, 1, staggered_reset=True) as iv:
                        body(iv)
                else:
                    for k in range(20):
                        body(k)
                # loop covered slots 2..61; finish 62, 63
                stage1b(V[62 % 3])                   # slot 62
                stage1a(63 * 128, WS[63 % 3])
                stage2(61 * 16, V[61 % 3])
                stage1b(V[63 % 3])                   # slot 63
                stage2(62 * 16, V[62 % 3])
                stage2(63 * 16, V[63 % 3])

            nc.sync.dma_start(out=out_ext.ap(), in_=outbuf)
    nc.compile()
    return nc


def _host_prep(inputs):
    worker_ids = np.asarray(inputs["worker_ids"]).astype(np.int64)
    project_ids = np.asarray(inputs["project_ids"]).astype(np.int64)
    worker_emb = np.asarray(inputs["worker_emb"], dtype=np.float32)
    project_emb = np.asarray(inputs["project_emb"], dtype=np.float32)
    W1 = np.asarray(inputs["W1"], dtype=np.float32)
    b1 = np.asarray(inputs["b1"], dtype=np.float32)
    W2 = np.asarray(inputs["W2"], dtype=np.float32)
    b2 = np.asarray(inputs["b2"], dtype=np.float32)

    table = project_emb[1:]
    G = (table @ W2).astype(np.float32)
    c = (table @ b2).astype(np.float32)
    G1 = _bf16(G)
    G2 = _bf16(G - G1.astype(np.float32))
    c1 = _bf16(c)
    c2 = _bf16(c - c1.astype(np.float32))
    tstk = np.zeros((122, NT), dtype=ml_dtypes.bfloat16)
    tstk[0:40, 0:NPROJ] = G1.T
    tstk[40:64, 0:NPROJ] = G2.T[0:24]
    tstk[64:104, 0:NPROJ] = G1.T
    tstk[104:120, 0:NPROJ] = G2.T[24:40]
    tstk[120, 0:NPROJ] = c1
    tstk[121, 0:NPROJ] = c2
    # pad column 2489: score = -20000 (never wins)
    tstk[120, NPROJ] = np.float32(-20000.0)

    def gtab16(emb, nrow):
        t = np.zeros((16, nrow), dtype=np.float32)
        t[0:EMB] = emb.T
        return t

    def widx_layout(ids_core):
        # [8 groups, 64 slots, 16 parts] -> [8, 16, 64] -> [128, 64]
        return ids_core.astype(np.int16).reshape(8, 64, 16).transpose(0, 2, 1).reshape(128, 64)

    shared = {
        "wtab16": gtab16(worker_emb, NW), "ptab16": gtab16(project_emb, NPTAB),
        "w1t": W1.T.astype(np.float32).copy(),
        "b1e": b1.reshape(HID, 1).astype(np.float32),
        "tstk": tstk,
    }
    in_maps = []
    for core in range(NCORES):
        sl = slice(core * BC, (core + 1) * BC)
        m = dict(shared)
        m["widx"] = widx_layout(worker_ids[sl])
        m["pidx"] = widx_layout(project_ids[sl])
        in_maps.append(m)
    return in_maps


def _decode(results):
    idx = np.zeros((B,), dtype=np.int64)
    for core in range(NCORES):
        o = results[core]["out"]          # [128, 16*NTILES] uint32
        for t in range(NTILES):
            rows = slice(core * BC + t * 128, core * BC + (t + 1) * 128)
            j = o[:, 16 * t].astype(np.int64)
            b = o[:, 16 * t + 8].astype(np.int64)
            idx[rows] = b * WGRID + j
    return (idx + 1).astype(np.int32).reshape(B, 1)


def kernel(**inputs):
    from concourse.bass_utils import run_bass_kernel_spmd
    in_maps = _host_prep(inputs)
    if "nc1" not in _cache:
        _cache["nc1"] = _build(L=1)
    res = run_bass_kernel_spmd(_cache["nc1"], in_maps, core_ids=list(range(NCORES)))
    return _decode(res.results)
